# revision 2
# baseline (speedup 1.0000x reference)
"""KanMxN fused B-spline kernel for 8 Trainium2 NeuronCores — v4.

Math: out[b,o] = sum_{i,p} basis[i,b,p] * coeff[i,p,o], degree-3 B-spline
basis on a uniform extended knot vector on [0,1] (n_params=16,
intervals=13). With t = 13*x+3, basis[i,b,p] = B(t-p), B the cardinal
cubic B-spline (support (0,4)).

Reformulation (exact):
 * middle translates p=4..11 use the folded form
   6B = r2^3 - 4*r1^3, r2 = relu(2-a), r1 = relu(1-a), a = |t-p-2|;
 * edge translates are absorbed into shared truncated-power rows via
   6B(s) = sum_k w_k relu(s-k)^3 = sum_k w_k relu(k-s)^3, w=[1,-4,6,-4,1]:
   p in {0..3} -> rows relu(m-t)^3, m=4..7; p in {12..15} -> rows
   relu(t-m)^3, m=12..15 — w-taps folded into coefficient panels.

Structural tricks:
 * Only the |.|-fold (a-pass) and the cube arguments (z-pass) need per-p
   constants; ALL other elementwise work is p-independent and runs as a
   few WIDE passes over [128, 4096] tiles covering 4 row-blocks each —
   amortizing instruction overhead and slashing instruction count (the
   original baseline was 335us, bottlenecked on 519 serialized DMA
   dispatches; v3 with per-block passes measured 67us, ACT-bound).
 * r2^3 and 4*r1^3 are fed to the PE as SEPARATE matmul rows with +-c6
   coefficient panels, so the final combine (and the factor -4, via
   r1c = relu(C-C*a), C = 4^(1/3)) costs zero vector-engine passes.
 * Odd powers need one relu only: s2*r2 = (2-a)^2*relu(2-a) = relu(2-a)^3,
   so Square passes read the unclamped affine directly.
 * Everything elementwise is fp16: PE fp16 matmuls run 1 cycle/row and
   DVE tensor_tensor gets the 2x packed mode (measured 657ns/1024 cols).

K grows to 24 row-blocks of 128 (folded p contributes 2 rows), 96
matmuls, 2 PSUM banks. Simulated end-to-end rel err with every
intermediate rounded to fp16: 5.3e-3 vs the 2e-2 gate.
"""

import numpy as np

N_IN, N_OUT, N_PARAMS, BATCH = 256, 256, 16, 4096
NCORES = 8
BL = BATCH // NCORES          # 512 batch per core
_C = float(np.float32(4.0) ** (np.float32(1.0) / 3.0))
_W = (1.0, -4.0, 6.0, -4.0, 1.0)

# halves: h=0 -> folded p 4..7, right cubes m 4..7 (z = (m-3) - 13x)
#         h=1 -> folded p 8..11, left cubes m 12..15 (z = 13x + (3-m))
_FOLD_P = [(4, 5, 6, 7), (8, 9, 10, 11)]
_CUBE_M = [(4, 5, 6, 7), (12, 13, 14, 15)]


# ------------------------------------------------------- walrus wait-limit post-pass
def _split_sync_waits(nc, max_waits=1):
    """CoreV3 CTRL instructions (Drain) accept few sem waits; hoist extras
    onto preceding NoOps on the same engine."""
    from concourse import mybir

    for f in nc.m.functions:
        for b in f.blocks:
            new_insts = []
            for inst in b.instructions:
                si = inst.sync_info
                if si is not None and si.on_wait and len(si.on_wait) > max_waits:
                    waits = list(si.on_wait)
                    extra, keep = waits[:-max_waits], waits[-max_waits:]
                    for ci in range(0, len(extra), max_waits):
                        chunk = extra[ci : ci + max_waits]
                        new_insts.append(
                            mybir.InstNoOp(
                                name=f"{inst.name}-ws{ci}",
                                engine=inst.engine,
                                ins=[],
                                outs=[],
                                sync_info=mybir.SyncInfo(on_wait=chunk, on_update=[]),
                            )
                        )
                    inst.sync_info = mybir.SyncInfo(
                        on_wait=keep, on_update=list(si.on_update or [])
                    )
                new_insts.append(inst)
            b.instructions = new_insts


# ---------------------------------------------------------------- program builder
_PROGRAM = {}


def _build_program():
    if "nc" in _PROGRAM:
        return _PROGRAM["nc"]
    import concourse.bass as bass
    import concourse.mybir as mybir
    from concourse import tile

    f32 = mybir.dt.float32
    f16 = mybir.dt.float16
    AF = mybir.ActivationFunctionType
    ALU = mybir.AluOpType

    nc = bass.Bass("TRN2", target_bir_lowering=True, debug=False, num_devices=NCORES)

    # const APs for ACT bias values: a-pass biases (1-p), 2.0, C
    bias_vals = {2.0, _C}
    for ps in _FOLD_P:
        for p in ps:
            bias_vals.add(float(1 - p))
    for bv in sorted(bias_vals):
        key = (f32, float(bv))
        if key not in nc.const_aps.aps:
            t = nc.alloc_sbuf_tensor(f"const-f32-{bv}", [128, 1], f32)
            nc.gpsimd.memset(t.ap(), float(bv))
            nc.const_aps.aps[key] = t.ap()
    nc.all_engine_barrier()

    x_d = nc.dram_tensor("x", [128, 2 * BL], f16, kind="ExternalInput").ap()
    # one d tensor per (half, kind): 4 blocks x 4 panels x 128 cols
    d_d = {}
    for h in range(2):
        for kind in ("c2", "cm", "T"):
            d_d[(h, kind)] = nc.dram_tensor(
                f"d{h}{kind}", [128, 2048], f16, kind="ExternalInput"
            ).ap()
    out_d = nc.dram_tensor("outT", [N_OUT, BL], f32, kind="ExternalOutput").ap()

    W = 4 * 1024  # half-tile: 4 row-blocks x 1024 cols

    with tile.TileContext(nc) as tc:
        with (
            tc.tile_pool(name="static", bufs=1) as static_pool,
            tc.tile_pool(name="big", bufs=2) as big_pool,
            tc.tile_pool(name="big1", bufs=1) as big1_pool,
            tc.tile_pool(name="prod", bufs=2) as prod_pool,
            tc.tile_pool(name="outp", bufs=1) as out_pool,
            tc.tile_pool(name="psum", bufs=1, space="PSUM") as psum_pool,
        ):
            x_sb = static_pool.tile([128, 2 * BL], f16, tag="x")
            nc.sync.dma_start(out=x_sb[:], in_=x_d[:])
            d_sb = {}
            for h in range(2):
                for kind in ("c2", "cm", "T"):
                    t = static_pool.tile(
                        [128, 2048], f16, tag=f"d{h}{kind}", name=f"d{h}{kind}"
                    )
                    nc.sync.dma_start(out=t[:], in_=d_d[(h, kind)][:])
                    d_sb[(h, kind)] = t

            ps = [psum_pool.tile([128, BL], f32, tag=f"ps{ot}", name=f"ps{ot}")
                  for ot in range(2)]

            HALF = W // 2  # 2048-col chunks (2 row-blocks each)

            def mm_chunk(h, kind, ck, src_tile, first=False, last=False):
                """Matmuls for row-blocks j = 2*ck, 2*ck+1 reading a
                [128, 2048] chunk tile. start/stop are per PSUM bank."""
                for jj in range(2):
                    j = 2 * ck + jj
                    for it in range(2):
                        rhs = src_tile[:, jj * 1024 + it * BL : jj * 1024 + (it + 1) * BL]
                        for ot in range(2):
                            c0 = ((j * 2 + it) * 2 + ot) * 128
                            lhsT = d_sb[(h, kind)][:, c0 : c0 + 128]
                            nc.tensor.matmul(
                                ps[ot][:], lhsT, rhs,
                                start=(first and jj == 0 and it == 0),
                                stop=(last and jj == 1 and it == 1),
                            )

            def chunked(op, out_t, *ins):
                for lo in (0, HALF):
                    op(out_t[:, lo : lo + HALF], *[t[:, lo : lo + HALF] for t in ins])

            # ---- stage 1: per-row-block constant passes + cube pipeline
            # (everything here is ACT-a-independent, so the DVE front-loads
            # the full cube chain for both halves while ACT grinds a/r2/s2)
            AAs, ZZs = [], []
            for h in range(2):
                AA = big_pool.tile([128, W], f16, tag="AA", name=f"AA{h}")
                for j, p in enumerate(_FOLD_P[h]):
                    nc.scalar.activation(
                        AA[:, j * 1024 : (j + 1) * 1024], x_sb[:], AF.Abs,
                        bias=float(1 - p), scale=13.0,
                    )
                AAs.append(AA)
                ZZ = big_pool.tile([128, W], f16, tag="ZZ", name=f"ZZ{h}")
                for j, m in enumerate(_CUBE_M[h]):
                    scale, b = (-13.0, float(m - 3)) if h == 0 else (13.0, float(3 - m))
                    nc.vector.tensor_scalar(
                        ZZ[:, j * 1024 : (j + 1) * 1024], x_sb[:], scale, b,
                        op0=ALU.mult, op1=ALU.add,
                    )
                ZZs.append(ZZ)

            first_done = [False]
            for h in range(2):
                ZZ = ZZs[h]
                for ck, lo in enumerate((0, HALF)):
                    zz = ZZ[:, lo : lo + HALF]
                    rZ = big1_pool.tile([128, HALF], f16, tag="rZ", name=f"rZ{h}{ck}")
                    nc.vector.tensor_scalar_max(rZ[:], zz, 0.0)
                    sZ = big1_pool.tile([128, HALF], f16, tag="sZ", name=f"sZ{h}{ck}")
                    nc.vector.tensor_mul(sZ[:], zz, zz)
                    T = prod_pool.tile([128, HALF], f16, tag="T", name=f"T{h}{ck}")
                    nc.vector.tensor_mul(T[:], sZ[:], rZ[:])
                    mm_chunk(h, "T", ck, T, first=not first_done[0])
                    first_done[0] = True

            # ---- stage 2: folded rows. ACT: r2 + s2 wide passes (h0 also
            # keeps r1 on ACT); h1's r1 moves to DVE TS pairs to shorten the
            # trailing ACT chain.
            for h in range(2):
                AA = AAs[h]
                r2 = big_pool.tile([128, W], f16, tag="r2", name=f"r2_{h}")
                chunked(lambda o, i: nc.scalar.activation(
                    o, i, AF.Relu, bias=2.0, scale=-1.0), r2, AA)
                s2 = big_pool.tile([128, W], f16, tag="s2", name=f"s2_{h}")
                chunked(lambda o, i: nc.scalar.activation(
                    o, i, AF.Square, bias=2.0, scale=-1.0), s2, AA)
                if h == 0:
                    r1 = big_pool.tile([128, W], f16, tag="r1", name=f"r1_{h}")
                    chunked(lambda o, i: nc.scalar.activation(
                        o, i, AF.Relu, bias=_C, scale=-_C), r1, AA)
                else:
                    # r1c = relu(C*r2 - C) == C*relu(r2-1) == C*relu(1-a)
                    r1 = big_pool.tile([128, W], f16, tag="r1", name=f"r1_{h}")
                    rt = big1_pool.tile([128, W], f16, tag="rt", name=f"rt_{h}")
                    chunked(lambda o, i: nc.vector.tensor_scalar(
                        o, i, _C, -_C, op0=ALU.mult, op1=ALU.add), rt, r2)
                    chunked(lambda o, i: nc.vector.tensor_scalar_max(o, i, 0.0),
                            r1, rt)

                for ck, lo in enumerate((0, HALF)):
                    c2 = prod_pool.tile([128, HALF], f16, tag="c2", name=f"c2{h}{ck}")
                    nc.vector.tensor_mul(
                        c2[:], s2[:, lo : lo + HALF], r2[:, lo : lo + HALF]
                    )
                    mm_chunk(h, "c2", ck, c2)
                    s1 = big1_pool.tile([128, HALF], f16, tag="s1", name=f"s1{h}{ck}")
                    nc.vector.tensor_mul(
                        s1[:], r1[:, lo : lo + HALF], r1[:, lo : lo + HALF]
                    )
                    cm = prod_pool.tile([128, HALF], f16, tag="cm", name=f"cm{h}{ck}")
                    nc.vector.tensor_mul(cm[:], s1[:], r1[:, lo : lo + HALF])
                    mm_chunk(h, "cm", ck, cm, last=(h == 1 and ck == 1))

            for ot in range(2):
                o_sb = out_pool.tile([128, BL], f32, tag=f"o{ot}", name=f"o{ot}")
                if ot == 0:
                    nc.scalar.copy(o_sb[:], ps[ot][:])
                else:
                    nc.vector.tensor_copy(o_sb[:], ps[ot][:])
                nc.sync.dma_start(out=out_d[ot * 128 : (ot + 1) * 128, :], in_=o_sb[:])

    _split_sync_waits(nc, max_waits=1)
    _PROGRAM["nc"] = nc
    return nc


# ---------------------------------------------------------------- host wrapper
def _pack_d(coeff):
    """Coefficient panels per (half, kind), fp16."""
    c6 = coeff.astype(np.float64) / 6.0  # [i, p, o]
    packs = {}
    for h in range(2):
        for kind in ("c2", "cm", "T"):
            buf = np.empty((128, 2048), dtype=np.float16)
            for j in range(4):
                if kind == "c2":
                    d = c6[:, _FOLD_P[h][j], :]
                elif kind == "cm":
                    d = -c6[:, _FOLD_P[h][j], :]
                else:
                    m = _CUBE_M[h][j]
                    ps = range(0, 4) if h == 0 else range(12, 16)
                    d = np.zeros((N_IN, N_OUT), dtype=np.float64)
                    for p in ps:
                        k = m - p
                        if 0 <= k <= 4:
                            d += _W[k] * c6[:, p, :]
                for it in range(2):
                    for ot in range(2):
                        c0 = ((j * 2 + it) * 2 + ot) * 128
                        buf[:, c0 : c0 + 128] = d[
                            it * 128 : (it + 1) * 128, ot * 128 : (ot + 1) * 128
                        ]
            packs[(h, kind)] = np.ascontiguousarray(buf)
    return packs


def kernel(x, coeff, _trace=False):
    x = np.ascontiguousarray(x, dtype=np.float32)
    coeff = np.ascontiguousarray(coeff, dtype=np.float32)
    assert x.shape == (N_IN, BATCH) and coeff.shape == (N_IN, N_PARAMS, N_OUT)

    from concourse.bass_utils import run_bass_kernel_spmd

    nc = _build_program()
    packs = _pack_d(coeff)

    in_maps = []
    for c in range(NCORES):
        xs = x[:, c * BL : (c + 1) * BL]  # [256, BL]
        x_sb = np.ascontiguousarray(
            np.concatenate([xs[:128, :], xs[128:, :]], axis=1).astype(np.float16)
        )
        im = {"x": x_sb}
        for (h, kind), buf in packs.items():
            im[f"d{h}{kind}"] = buf
        in_maps.append(im)

    res = run_bass_kernel_spmd(nc, in_maps, list(range(NCORES)), trace=_trace)
    out = np.empty((BATCH, N_OUT), dtype=np.float32)
    for c in range(NCORES):
        out[c * BL : (c + 1) * BL, :] = res.results[c]["outT"].T
    if _trace:
        return out, res
    return out
